# revision 19
# baseline (speedup 1.0000x reference)
"""Butterfly sparse-attention MLP kernel for 8 Trainium2 NeuronCores.

Computation (from the reference):
    attn = (w1.T @ w2.T) * sparse_mask          # [4096 s, 4096 t]
    y    = gelu(x @ attn + b2)                  # [8, 768, 4096]

sparse_mask is banded: mask[s, t] == 0 whenever |s - t| > 133.  Each core
owns a 512-wide t-block and needs only a 784-wide s-window around it
(133 before + 512 + 139 after, the last 6 rows zero padding).  Per
t-subtile of 128, only 394 s-rows are in band, so phase B contracts over
3 full 128-chunks plus a 16-row chunk, and phase A computes only the
exact in-band t-range of each attn chunk (the rest of each attn SBUF
tile is memset to zero).

Sharding: tensor-parallel over t (8 blocks of 512).  All per-core variation
is in the input data (windows are zero-padded at the edges; mask zeros make
padded contributions exactly zero), so one SPMD BIR serves all 8 cores.

DMA model (measured): per-dma_start rate rises with transfer size (~310
GB/s at 0.5 MB, ~395 GB/s at 1 MB) and multiple queues do NOT add up —
the ~410 GB/s fabric cap is shared.  So all weights ride ONE queue (sync)
as ~1 MB pieces, interleaved w1/w2 in phase A's consumption order; x rides
the gpsimd ring (first-group slices trickled early via deps, the rest
sequenced behind them); the scalar engine does nothing but gelu so
activations never stall; y stores go back on sync once weights finish.
"""

import numpy as np

B, T, D = 8, 768, 4096
N = B * T            # 6144 rows of x
NCORES = 8
TB = 512             # t-columns per core
P = 128
M0 = 133             # s-window starts this far before the t-block
SW = 784             # s-window width (133 + 512 + 133 + 6 pad)
NCH = 7              # s-chunks: 6 full + one 16-row chunk
LASTROWS = 16        # rows in chunk 6 (10 in-band + 6 pad)
DCH = D // P         # 32 d-chunks (contraction of phase A)
NQ = TB // P         # 4 t-subtiles per core
GN = 2048            # n-group width in phase B
NG = N // GN         # 3 n-groups
BANDCH = 4           # s-chunks feeding one t-subtile (covers +-133 band)
W1SPLIT = (2, 5, 5, 5, 5, 5, 5)   # w1 d-chunks per DMA piece
W2SPLIT = (4, 8, 8, 8, 4)         # w2 d-chunks per DMA piece

# Exact in-band t-range [lo_j, hi_j) of attn chunk j (window coords with
# M0 = 133: rows of chunk j are s - t0 + 133 in [128j, 128j+rows)).
BAND_LO = (0, 0, 0, 118, 246, 374, 502)
BAND_HI = (128, 256, 384, 512, 512, 512, 512)
BAND_W = tuple(h - l for l, h in zip(BAND_LO, BAND_HI))
MOFF = tuple(sum(BAND_W[:j]) for j in range(NCH))  # mask col offsets
MW = sum(BAND_W)  # 1576

_NC = None


def _build_module():
    from concourse import bacc, bass, mybir, tile
    from concourse.tile_rust import add_dep_helper

    f32 = mybir.dt.float32
    f16 = mybir.dt.float16
    PSUM = bass.MemorySpace.PSUM

    nc = bacc.Bacc("TRN2", target_bir_lowering=False, debug=False)
    xT_d = nc.declare_dram_parameter("xT_s", [NCH - 1, P, N], f16, isOutput=False)
    x6_d = nc.declare_dram_parameter("x6_s", [LASTROWS, N], f16, isOutput=False)
    w1_d = nc.declare_dram_parameter("w1_s", [P, DCH * SW], f16, isOutput=False)
    w2_d = nc.declare_dram_parameter("w2_s", [P, DCH * TB], f16, isOutput=False)
    mask_d = nc.declare_dram_parameter("mask_s", [P, MW], f16, isOutput=False)
    b2_d = nc.declare_dram_parameter("b2c_s", [P, NQ], f32, isOutput=False)
    yT_d = nc.declare_dram_parameter("yT_s", [TB, N], f16, isOutput=True)

    with tile.TileContext(nc) as tc:
        with (
            tc.tile_pool(name="const", bufs=1) as cpool,
            tc.tile_pool(name="attn", bufs=1) as apool,
            tc.tile_pool(name="xp", bufs=NCH) as xp,
            tc.tile_pool(name="yp", bufs=4) as yp,
        ):
            b2_t = cpool.tile([P, NQ], f32)
            m_t = cpool.tile([P, MW], f16)

            # attn tiles are full 512 wide; zero them so phase B's 128-wide
            # stationary reads see zeros outside the exact band.
            attn_sb = []
            for j in range(NCH):
                a_t = apool.tile([P, TB], f16, name=f"attn_sb{j}")
                nc.vector.memset(a_t[:], 0.0)
                attn_sb.append(a_t)

            w_insts = []

            # ---- Phase A: attn[s, t] = (w1.T @ w2T) * mask on the band ----
            with (
                tc.tile_pool(name="w1p", bufs=1) as w1p,
                tc.tile_pool(name="w2p", bufs=1) as w2p,
                tc.tile_pool(name="psA", bufs=1, space=PSUM) as psA,
            ):
                w1_t = w1p.tile([P, DCH * SW], f16)
                w2_t = w2p.tile([P, DCH * TB], f16)

                # One weight ring on sync, pieces ordered so (w1 chunks <= k,
                # w2 chunks <= k) arrive just ahead of the PE's k loop.
                s1 = [int(v) for v in np.cumsum((0,) + W1SPLIT)]
                s2 = [int(v) for v in np.cumsum((0,) + W2SPLIT)]

                def w1_piece(pi):
                    w_insts.append(nc.sync.dma_start(
                        w1_t[:, s1[pi] * SW:s1[pi + 1] * SW],
                        w1_d[:, s1[pi] * SW:s1[pi + 1] * SW]))

                def w2_piece(pi):
                    w_insts.append(nc.sync.dma_start(
                        w2_t[:, s2[pi] * TB:s2[pi + 1] * TB],
                        w2_d[:, s2[pi] * TB:s2[pi + 1] * TB]))

                w2_piece(0)            # k0-3
                w1_piece(0)            # k0-1
                w1_piece(1)            # k2-6
                w2_piece(1)            # k4-11
                w1_piece(2)            # k7-11
                w1_piece(3)            # k12-16
                w2_piece(2)            # k12-19
                w1_piece(4)            # k17-21
                w2_piece(3)            # k20-27
                w1_piece(5)            # k22-26
                w1_piece(6)            # k27-31
                w2_piece(4)            # k28-31
                w_insts.append(nc.sync.dma_start(m_t[:], mask_d[:]))
                w_insts.append(nc.sync.dma_start(b2_t[:], b2_d[:]))

                attn_ps = [
                    psA.tile([P, BAND_W[j]], f32, name=f"attn_ps{j}")
                    for j in range(NCH)
                ]
                for k in range(DCH):
                    w1row = w1_t[:, k * SW:(k + 1) * SW]
                    w2row = w2_t[:, k * TB:(k + 1) * TB]
                    # last k in j-ascending order so the mask-muls (and
                    # phase B's first subtile) unblock in consumption order
                    jorder = range(NCH) if k == DCH - 1 else (3, 2, 4, 1, 5, 0, 6)
                    for j in jorder:
                        scols = LASTROWS if j == 6 else P
                        nc.tensor.matmul(
                            attn_ps[j][0:scols, 0:BAND_W[j]],
                            w1row[:, j * P:j * P + scols],
                            w2row[:, BAND_LO[j]:BAND_HI[j]],
                            start=(k == 0),
                            stop=(k == DCH - 1),
                        )
                for j in range(NCH):
                    rows = LASTROWS if j == 6 else P
                    nc.vector.tensor_mul(
                        attn_sb[j][0:rows, BAND_LO[j]:BAND_HI[j]],
                        attn_ps[j][0:rows, 0:BAND_W[j]],
                        m_t[0:rows, MOFF[j]:MOFF[j] + BAND_W[j]],
                    )

            # x rides the gpsimd ring: first-group slices of each chunk are
            # trickled during the weight phase (deps below stall the gpsimd
            # trigger stream, so everything later queues behind them); the
            # remaining 4096-wide slices follow in ring order.
            # w_insts: [w2p0, w1p0, w1p1, w2p1, w1p2, w1p3, w2p2, w1p4,
            #           w2p3, w1p5, w1p6, w2p4, mask, b2]
            x_t = [None] * NCH
            g0_gate = {0: 2, 1: 3, 2: 5, 3: 6, 4: 8, 5: 9, 6: 10}
            for j in range(NCH):
                rows = LASTROWS if j == 6 else P
                xt = xp.tile([rows, N], f16, name=f"x_t{j}", tag="x_t")
                src = x6_d if j == 6 else xT_d[j]
                xi = nc.gpsimd.dma_start(xt[:, 0:GN], src[:, 0:GN])
                add_dep_helper(xi.ins, w_insts[g0_gate[j]].ins, sync=True,
                               reason="pace x g0 behind weights")
                x_t[j] = xt
            for j in range(NCH):
                src = x6_d if j == 6 else xT_d[j]
                nc.gpsimd.dma_start(x_t[j][:, GN:N], src[:, GN:N])

            # ---- Phase B: yT[t, n] = gelu(attn.T @ xT + b2) on the band ----
            with tc.tile_pool(name="psB", bufs=4, space=PSUM) as psB:
                for g in range(NG):
                    for q in range(NQ):
                        for h in range(2):
                            nsl = slice(g * GN + h * 1024,
                                        g * GN + (h + 1) * 1024)
                            y_ps = psB.tile([P, 1024], f32, name="y_ps",
                                            tag="y_ps")
                            for hh in range(2):
                                osl = slice(hh * 512, (hh + 1) * 512)
                                xsl = slice(nsl.start + hh * 512,
                                            nsl.start + (hh + 1) * 512)
                                for c in range(BANDCH):
                                    j = q + c
                                    rows = LASTROWS if j == 6 else P
                                    nc.tensor.matmul(
                                        y_ps[:, osl],
                                        attn_sb[j][0:rows, q * P:(q + 1) * P],
                                        x_t[j][0:rows, xsl],
                                        start=(c == 0),
                                        stop=(c == BANDCH - 1),
                                    )
                            y_sb = yp.tile([P, 1024], f16, name="y_sb",
                                           tag="y_sb")
                            last = (g, q, h) == (NG - 1, NQ - 1, 1)
                            parts = (2 if last else 1)
                            for pp in range(parts):
                                lo = pp * 1024 // parts
                                hi = (pp + 1) * 1024 // parts
                                base = g * GN + h * 1024
                                nc.scalar.activation(
                                    y_sb[:, lo:hi],
                                    y_ps[:, lo:hi],
                                    mybir.ActivationFunctionType.Gelu,
                                    bias=b2_t[:, q:q + 1],
                                    scale=1.0,
                                )
                                nc.sync.dma_start(
                                    yT_d[q * P:(q + 1) * P,
                                         base + lo:base + hi],
                                    y_sb[:, lo:hi],
                                )

    nc.compile()
    nc.finalize()
    return nc


def _get_nc():
    global _NC
    if _NC is None:
        _NC = _build_module()
    return _NC


def prepare_in_maps(x, w1, w2, b2, sparse_mask):
    x = np.asarray(x, dtype=np.float32)
    w1 = np.asarray(w1, dtype=np.float32)
    w2 = np.asarray(w2, dtype=np.float32)
    b2 = np.asarray(b2, dtype=np.float32)
    sparse_mask = np.asarray(sparse_mask, dtype=np.float32)

    xT = np.ascontiguousarray(x.reshape(N, D).T.astype(np.float16))   # [s, n]
    w2T = np.ascontiguousarray(w2.T.astype(np.float16))               # [d, t]

    # Zero-pad the s axis (133 left, 139 right) so every core's window is
    # a plain slice; mask zeros make the padded rows contribute nothing.
    PADL, PADR = M0, SW - TB - M0
    xT_pad = np.zeros((D + PADL + PADR, N), dtype=np.float16)
    xT_pad[PADL:PADL + D] = xT
    w1_pad = np.zeros((D, D + PADL + PADR), dtype=np.float16)
    w1_pad[:, PADL:PADL + D] = w1.astype(np.float16)
    mask_pad = np.zeros((D + PADL + PADR, D), dtype=np.float16)
    mask_pad[PADL:PADL + D] = sparse_mask.astype(np.float16)

    in_maps = []
    for i in range(NCORES):
        s0 = i * TB           # window start in padded coords
        t0 = i * TB
        win = xT_pad[s0:s0 + SW]                          # [SW, N]
        # weights packed so each partition's 32 chunks are contiguous:
        # w1_s[p, k*SW + c] = w1_pad[128k + p, s0 + c]
        w1win = w1_pad[:, s0:s0 + SW]                     # [D, SW]
        w1_s = np.ascontiguousarray(
            w1win.reshape(DCH, P, SW).transpose(1, 0, 2).reshape(P, DCH * SW))
        w2win = w2T[:, t0:t0 + TB]                        # [D, TB]
        w2_s = np.ascontiguousarray(
            w2win.reshape(DCH, P, TB).transpose(1, 0, 2).reshape(P, DCH * TB))
        # mask packed per chunk at its exact band: [128, MW]
        mask_s = np.zeros((P, MW), dtype=np.float16)
        mwin = mask_pad[s0:s0 + SW, t0:t0 + TB]           # [SW, TB]
        for j in range(NCH):
            rows = LASTROWS if j == NCH - 1 else P
            mask_s[0:rows, MOFF[j]:MOFF[j] + BAND_W[j]] = (
                mwin[j * P:j * P + rows, BAND_LO[j]:BAND_HI[j]])
        in_maps.append({
            "xT_s": np.ascontiguousarray(
                win[:(NCH - 1) * P].reshape(NCH - 1, P, N)),
            "x6_s": np.ascontiguousarray(win[(NCH - 1) * P:]),
            "w1_s": w1_s,
            "w2_s": w2_s,
            "mask_s": mask_s,
            "b2c_s": np.ascontiguousarray(b2[t0:t0 + TB].reshape(NQ, P).T),
        })
    return in_maps


def assemble(results):
    out = np.empty((N, D), dtype=np.float32)
    for i in range(NCORES):
        out[:, i * TB:(i + 1) * TB] = results[i]["yT_s"].T.astype(np.float32)
    return out.reshape(B, T, D)


def _band_ok(sparse_mask):
    """The Bass kernel only computes attn on the exact per-chunk bands;
    verify every mask nonzero falls inside that region."""
    s_idx, t_idx = np.nonzero(np.asarray(sparse_mask) != 0)
    if len(s_idx) == 0:
        return True
    w0 = (t_idx // TB) * TB - M0              # per-core s-window start
    r = s_idx - w0                            # row in window coords
    if not np.all((r >= 0) & (r < SW - 6)):
        return False
    j = r // P
    tp = t_idx % TB
    lo = np.asarray(BAND_LO)[j]
    hi = np.asarray(BAND_HI)[j]
    return bool(np.all((tp >= lo) & (tp < hi)))


def _reference_fallback(x, w1, w2, b2, sparse_mask):
    import jax
    import jax.numpy as jnp

    cpu = jax.devices("cpu")[0]
    with jax.default_device(cpu):
        attn = jnp.einsum("ds,td->st", jnp.asarray(w1), jnp.asarray(w2))
        attn = attn * jnp.asarray(sparse_mask)
        y = jnp.einsum("bds,st->bdt", jnp.asarray(x), attn) + jnp.asarray(b2)
        return np.asarray(jax.nn.gelu(y, approximate=False), dtype=np.float32)


def kernel(x, w1, w2, b2, sparse_mask):
    import time

    from concourse.bass_utils import run_bass_kernel_spmd

    if (np.shape(x) != (B, T, D) or np.shape(w1) != (D, D)
            or np.shape(w2) != (D, D) or np.shape(b2) != (D,)
            or np.shape(sparse_mask) != (D, D) or not _band_ok(sparse_mask)):
        return _reference_fallback(x, w1, w2, b2, sparse_mask)

    in_maps = prepare_in_maps(x, w1, w2, b2, sparse_mask)
    nc = _get_nc()
    last_err = None
    for attempt in range(3):
        try:
            res = run_bass_kernel_spmd(nc, in_maps, list(range(NCORES)))
            return assemble(res.results)
        except Exception as e:  # transient NRT/device errors: retry
            last_err = e
            time.sleep(2.0 * (attempt + 1))
    raise last_err


# revision 24
# speedup vs baseline: 1.1471x; 1.1471x over previous
"""Butterfly sparse-attention MLP kernel for 8 Trainium2 NeuronCores.

Computation (from the reference):
    attn = (w1.T @ w2.T) * sparse_mask          # [4096 s, 4096 t]
    y    = gelu(x @ attn + b2)                  # [8, 768, 4096]

sparse_mask is banded: mask[s, t] == 0 whenever |s - t| > 133.  Each core
owns a 512-wide t-block and needs only a 784-wide s-window around it
(133 before + 512 + 139 after, the last 6 rows zero padding).  Per
t-subtile of 128, only 394 s-rows are in band, so phase B contracts over
3 full 128-chunks plus a 16-row chunk, and phase A computes only the
exact in-band t-range of each attn chunk (the rest of each attn SBUF
tile is memset to zero).

Sharding: tensor-parallel over t (8 blocks of 512).  All per-core variation
is in the input data (windows are zero-padded at the edges; mask zeros make
padded contributions exactly zero), so one SPMD BIR serves all 8 cores.

DMA model (measured): per-dma_start rate rises with transfer size (~310
GB/s at 0.5 MB, ~395 GB/s at 1 MB) and multiple queues do NOT add up —
the ~410 GB/s fabric cap is shared.  So all weights ride ONE queue (sync)
as ~1 MB pieces, interleaved w1/w2 in phase A's consumption order; x rides
the gpsimd ring (first-group slices trickled early via deps, the rest
sequenced behind them); the scalar engine does nothing but gelu so
activations never stall; y stores go back on sync once weights finish.
"""

import numpy as np

B, T, D = 8, 768, 4096
N = B * T            # 6144 rows of x
NCORES = 8
TB = 512             # t-columns per core
P = 128
M0 = 133             # s-window starts this far before the t-block
SW = 784             # s-window width (133 + 512 + 133 + 6 pad)
NCH = 7              # s-chunks: 6 full + one 16-row chunk
LASTROWS = 16        # rows in chunk 6 (10 in-band + 6 pad)
DCH = D // P         # 32 d-chunks (contraction of phase A)
NQ = TB // P         # 4 t-subtiles per core
GN = 2048            # n-group width in phase B
NG = N // GN         # 3 n-groups
BANDCH = 4           # s-chunks feeding one t-subtile (covers +-133 band)
W1SPLIT = (2, 5, 5, 5, 5, 5, 5)   # w1 d-chunks per DMA piece

# Exact in-band t-range [lo_j, hi_j) of attn chunk j (window coords with
# M0 = 133: rows of chunk j are s - t0 + 133 in [128j, 128j+rows)).
BAND_LO = (0, 0, 0, 118, 246, 374, 502)
BAND_HI = (128, 256, 384, 512, 512, 512, 512)
BAND_W = tuple(h - l for l, h in zip(BAND_LO, BAND_HI))
MOFF = tuple(sum(BAND_W[:j]) for j in range(NCH))  # mask col offsets
MW = sum(BAND_W)  # 1576

_NC = None


def _build_module():
    from concourse import bacc, bass, mybir, tile
    from concourse.tile_rust import add_dep_helper

    f32 = mybir.dt.float32
    f16 = mybir.dt.float16
    PSUM = bass.MemorySpace.PSUM

    nc = bacc.Bacc("TRN2", target_bir_lowering=False, debug=False)
    xT_d = nc.declare_dram_parameter("xT_s", [NCH - 1, P, N], f16, isOutput=False)
    x6_d = nc.declare_dram_parameter("x6_s", [LASTROWS, N], f16, isOutput=False)
    w1_d = nc.declare_dram_parameter("w1_s", [P, DCH * SW], f16, isOutput=False)
    w2_d = nc.declare_dram_parameter("w2_s", [P, DCH * TB], f16, isOutput=False)
    mask_d = nc.declare_dram_parameter("mask_s", [P, MW], f16, isOutput=False)
    b2_d = nc.declare_dram_parameter("b2c_s", [P, NQ], f32, isOutput=False)
    yT_d = nc.declare_dram_parameter("yT_s", [TB, N], f16, isOutput=True)

    with tile.TileContext(nc) as tc:
        with (
            tc.tile_pool(name="const", bufs=1) as cpool,
            tc.tile_pool(name="attn", bufs=1) as apool,
            tc.tile_pool(name="xp", bufs=NG * NCH) as xp,
            tc.tile_pool(name="yp", bufs=4) as yp,
        ):
            b2_t = cpool.tile([P, NQ], f32)
            m_t = cpool.tile([P, MW], f16)

            # attn tiles are full 512 wide; zero them so phase B's 128-wide
            # stationary reads see zeros outside the exact band.
            attn_sb = []
            for j in range(NCH):
                a_t = apool.tile([P, TB], f16, name=f"attn_sb{j}")
                nc.vector.memset(a_t[:], 0.0)
                attn_sb.append(a_t)

            w_insts = []

            # ---- Phase A: attn[s, t] = (w1.T @ w2T) * mask on the band ----
            # Every DMA piece gets its OWN tile: consecutive DMAs into one
            # tile serialize (per-tile dependency tracking) and each pays a
            # ~2us completion bubble, killing the stream rate.
            with (
                tc.tile_pool(name="w1p", bufs=len(W1SPLIT)) as w1p,
                tc.tile_pool(name="w2p", bufs=DCH // 4) as w2p,
                tc.tile_pool(name="psA", bufs=1, space=PSUM) as psA,
            ):
                s1 = [int(v) for v in np.cumsum((0,) + W1SPLIT)]
                w1_ts = []
                w2_ts = []
                w1i = 0
                w2i = 0

                def w1_piece():
                    nonlocal w1i
                    t = w1p.tile([P, W1SPLIT[w1i] * SW], f16,
                                 name=f"w1_{w1i}", tag="w1")
                    w_insts.append(nc.sync.dma_start(
                        t[:], w1_d[:, s1[w1i] * SW:s1[w1i + 1] * SW]))
                    w1_ts.append(t)
                    w1i += 1

                def w2_piece():
                    nonlocal w2i
                    t = w2p.tile([P, 4 * TB], f16, name=f"w2_{w2i}", tag="w2")
                    w_insts.append(nc.scalar.dma_start(
                        t[:], w2_d[:, w2i * 4 * TB:(w2i + 1) * 4 * TB]))
                    w2_ts.append(t)
                    w2i += 1

                # issue order = w_insts index used by the x gates below
                w2_piece()             # 0: k0-3
                w1_piece()             # 1: k0-1
                w1_piece()             # 2: k2-6
                w2_piece()             # 3: k4-7
                w2_piece()             # 4: k8-11
                w1_piece()             # 5: k7-11
                w2_piece()             # 6: k12-15
                w1_piece()             # 7: k12-16
                w2_piece()             # 8: k16-19
                w1_piece()             # 9: k17-21
                w2_piece()             # 10: k20-23
                w1_piece()             # 11: k22-26
                w2_piece()             # 12: k24-27
                w1_piece()             # 13: k27-31
                w2_piece()             # 14: k28-31
                w_insts.append(nc.scalar.dma_start(m_t[:], mask_d[:]))
                w_insts.append(nc.scalar.dma_start(b2_t[:], b2_d[:]))

                def w1row_of(k):
                    pi = next(i for i in range(len(W1SPLIT))
                              if s1[i] <= k < s1[i + 1])
                    off = (k - s1[pi]) * SW
                    return w1_ts[pi][:, off:off + SW]

                attn_ps = [
                    psA.tile([P, BAND_W[j]], f32, name=f"attn_ps{j}")
                    for j in range(NCH)
                ]
                for k in range(DCH):
                    w1row = w1row_of(k)
                    w2row = w2_ts[k // 4][:, (k % 4) * TB:(k % 4 + 1) * TB]
                    # last k in j-ascending order so the mask-muls (and
                    # phase B's first subtile) unblock in consumption order
                    jorder = range(NCH) if k == DCH - 1 else (3, 2, 4, 1, 5, 0, 6)
                    for j in jorder:
                        scols = LASTROWS if j == 6 else P
                        nc.tensor.matmul(
                            attn_ps[j][0:scols, 0:BAND_W[j]],
                            w1row[:, j * P:j * P + scols],
                            w2row[:, BAND_LO[j]:BAND_HI[j]],
                            start=(k == 0),
                            stop=(k == DCH - 1),
                        )
                for j in range(NCH):
                    rows = LASTROWS if j == 6 else P
                    nc.vector.tensor_mul(
                        attn_sb[j][0:rows, BAND_LO[j]:BAND_HI[j]],
                        attn_ps[j][0:rows, 0:BAND_W[j]],
                        m_t[0:rows, MOFF[j]:MOFF[j] + BAND_W[j]],
                    )

            # x rides the gpsimd ring as per-(group, chunk) tiles.  Group-0
            # chunks are trickled during the weight phase via engine-stall
            # deps on the weight DMAs (the gpsimd trigger stream is FIFO,
            # so each gate also delays everything after it); groups 1-2
            # queue up behind them and drain during phase B.
            x_t = [[None] * NCH for _ in range(NG)]
            gates = {(0, 0): 2, (0, 1): 4, (0, 2): 6, (0, 3): 8,
                     (0, 4): 10, (0, 5): 12, (0, 6): 13, (1, 0): 14}
            for g in range(NG):
                for j in range(NCH):
                    rows = LASTROWS if j == 6 else P
                    xt = xp.tile([rows, GN], f16, name="x_t", tag="x_t")
                    src = x6_d if j == 6 else xT_d[j]
                    xi = nc.gpsimd.dma_start(
                        xt[:], src[:, g * GN:(g + 1) * GN])
                    gate = gates.get((g, j))
                    if gate is not None:
                        add_dep_helper(
                            xi.ins, w_insts[gate].ins, sync=True,
                            reason="pace x behind weights")
                    x_t[g][j] = xt

            # ---- Phase B: yT[t, n] = gelu(attn.T @ xT + b2) on the band ----
            with tc.tile_pool(name="psB", bufs=4, space=PSUM) as psB:
                for g in range(NG):
                    for q in range(NQ):
                        for h in range(2):
                            y_ps = psB.tile([P, 1024], f32, name="y_ps",
                                            tag="y_ps")
                            for hh in range(2):
                                osl = slice(hh * 512, (hh + 1) * 512)
                                xsl = slice(h * 1024 + hh * 512,
                                            h * 1024 + (hh + 1) * 512)
                                for c in range(BANDCH):
                                    j = q + c
                                    rows = LASTROWS if j == 6 else P
                                    nc.tensor.matmul(
                                        y_ps[:, osl],
                                        attn_sb[j][0:rows, q * P:(q + 1) * P],
                                        x_t[g][j][0:rows, xsl],
                                        start=(c == 0),
                                        stop=(c == BANDCH - 1),
                                    )
                            y_sb = yp.tile([P, 1024], f16, name="y_sb",
                                           tag="y_sb")
                            last = (g, q, h) == (NG - 1, NQ - 1, 1)
                            parts = (2 if last else 1)
                            for pp in range(parts):
                                lo = pp * 1024 // parts
                                hi = (pp + 1) * 1024 // parts
                                base = g * GN + h * 1024
                                nc.scalar.activation(
                                    y_sb[:, lo:hi],
                                    y_ps[:, lo:hi],
                                    mybir.ActivationFunctionType.Gelu,
                                    bias=b2_t[:, q:q + 1],
                                    scale=1.0,
                                )
                                nc.sync.dma_start(
                                    yT_d[q * P:(q + 1) * P,
                                         base + lo:base + hi],
                                    y_sb[:, lo:hi],
                                )

    nc.compile()
    nc.finalize()
    return nc


def _get_nc():
    global _NC
    if _NC is None:
        _NC = _build_module()
    return _NC


def prepare_in_maps(x, w1, w2, b2, sparse_mask):
    x = np.asarray(x, dtype=np.float32)
    w1 = np.asarray(w1, dtype=np.float32)
    w2 = np.asarray(w2, dtype=np.float32)
    b2 = np.asarray(b2, dtype=np.float32)
    sparse_mask = np.asarray(sparse_mask, dtype=np.float32)

    xT = np.ascontiguousarray(x.reshape(N, D).T.astype(np.float16))   # [s, n]
    w2T = np.ascontiguousarray(w2.T.astype(np.float16))               # [d, t]

    # Zero-pad the s axis (133 left, 139 right) so every core's window is
    # a plain slice; mask zeros make the padded rows contribute nothing.
    PADL, PADR = M0, SW - TB - M0
    xT_pad = np.zeros((D + PADL + PADR, N), dtype=np.float16)
    xT_pad[PADL:PADL + D] = xT
    w1_pad = np.zeros((D, D + PADL + PADR), dtype=np.float16)
    w1_pad[:, PADL:PADL + D] = w1.astype(np.float16)
    mask_pad = np.zeros((D + PADL + PADR, D), dtype=np.float16)
    mask_pad[PADL:PADL + D] = sparse_mask.astype(np.float16)

    in_maps = []
    for i in range(NCORES):
        s0 = i * TB           # window start in padded coords
        t0 = i * TB
        win = xT_pad[s0:s0 + SW]                          # [SW, N]
        # weights packed so each partition's 32 chunks are contiguous:
        # w1_s[p, k*SW + c] = w1_pad[128k + p, s0 + c]
        w1win = w1_pad[:, s0:s0 + SW]                     # [D, SW]
        w1_s = np.ascontiguousarray(
            w1win.reshape(DCH, P, SW).transpose(1, 0, 2).reshape(P, DCH * SW))
        w2win = w2T[:, t0:t0 + TB]                        # [D, TB]
        w2_s = np.ascontiguousarray(
            w2win.reshape(DCH, P, TB).transpose(1, 0, 2).reshape(P, DCH * TB))
        # mask packed per chunk at its exact band: [128, MW]
        mask_s = np.zeros((P, MW), dtype=np.float16)
        mwin = mask_pad[s0:s0 + SW, t0:t0 + TB]           # [SW, TB]
        for j in range(NCH):
            rows = LASTROWS if j == NCH - 1 else P
            mask_s[0:rows, MOFF[j]:MOFF[j] + BAND_W[j]] = (
                mwin[j * P:j * P + rows, BAND_LO[j]:BAND_HI[j]])
        in_maps.append({
            "xT_s": np.ascontiguousarray(
                win[:(NCH - 1) * P].reshape(NCH - 1, P, N)),
            "x6_s": np.ascontiguousarray(win[(NCH - 1) * P:]),
            "w1_s": w1_s,
            "w2_s": w2_s,
            "mask_s": mask_s,
            "b2c_s": np.ascontiguousarray(b2[t0:t0 + TB].reshape(NQ, P).T),
        })
    return in_maps


def assemble(results):
    out = np.empty((N, D), dtype=np.float32)
    for i in range(NCORES):
        out[:, i * TB:(i + 1) * TB] = results[i]["yT_s"].T.astype(np.float32)
    return out.reshape(B, T, D)


def _band_ok(sparse_mask):
    """The Bass kernel only computes attn on the exact per-chunk bands;
    verify every mask nonzero falls inside that region."""
    s_idx, t_idx = np.nonzero(np.asarray(sparse_mask) != 0)
    if len(s_idx) == 0:
        return True
    w0 = (t_idx // TB) * TB - M0              # per-core s-window start
    r = s_idx - w0                            # row in window coords
    if not np.all((r >= 0) & (r < SW - 6)):
        return False
    j = r // P
    tp = t_idx % TB
    lo = np.asarray(BAND_LO)[j]
    hi = np.asarray(BAND_HI)[j]
    return bool(np.all((tp >= lo) & (tp < hi)))


def _reference_fallback(x, w1, w2, b2, sparse_mask):
    import jax
    import jax.numpy as jnp

    cpu = jax.devices("cpu")[0]
    with jax.default_device(cpu):
        attn = jnp.einsum("ds,td->st", jnp.asarray(w1), jnp.asarray(w2))
        attn = attn * jnp.asarray(sparse_mask)
        y = jnp.einsum("bds,st->bdt", jnp.asarray(x), attn) + jnp.asarray(b2)
        return np.asarray(jax.nn.gelu(y, approximate=False), dtype=np.float32)


def kernel(x, w1, w2, b2, sparse_mask):
    import time

    from concourse.bass_utils import run_bass_kernel_spmd

    if (np.shape(x) != (B, T, D) or np.shape(w1) != (D, D)
            or np.shape(w2) != (D, D) or np.shape(b2) != (D,)
            or np.shape(sparse_mask) != (D, D) or not _band_ok(sparse_mask)):
        return _reference_fallback(x, w1, w2, b2, sparse_mask)

    in_maps = prepare_in_maps(x, w1, w2, b2, sparse_mask)
    nc = _get_nc()
    last_err = None
    for attempt in range(3):
        try:
            res = run_bass_kernel_spmd(nc, in_maps, list(range(NCORES)))
            return assemble(res.results)
        except Exception as e:  # transient NRT/device errors: retry
            last_err = e
            time.sleep(2.0 * (attempt + 1))
    raise last_err


# revision 25
# speedup vs baseline: 1.1968x; 1.0434x over previous
"""Butterfly sparse-attention MLP kernel for 8 Trainium2 NeuronCores.

Computation (from the reference):
    attn = (w1.T @ w2.T) * sparse_mask          # [4096 s, 4096 t]
    y    = gelu(x @ attn + b2)                  # [8, 768, 4096]

sparse_mask is banded: mask[s, t] == 0 whenever |s - t| > 133.  Each core
owns a 512-wide t-block and needs only a 784-wide s-window around it
(133 before + 512 + 139 after, the last 6 rows zero padding).  Per
t-subtile of 128, only 394 s-rows are in band, so phase B contracts over
3 full 128-chunks plus a 16-row chunk, and phase A computes only the
exact in-band t-range of each attn chunk (the rest of each attn SBUF
tile is memset to zero).

Sharding: tensor-parallel over t (8 blocks of 512).  All per-core variation
is in the input data (windows are zero-padded at the edges; mask zeros make
padded contributions exactly zero), so one SPMD BIR serves all 8 cores.

DMA model (measured): per-dma_start rate rises with transfer size (~310
GB/s at 0.5 MB, ~395 GB/s at 1 MB) and multiple queues do NOT add up —
the ~410 GB/s fabric cap is shared.  So all weights ride ONE queue (sync)
as ~1 MB pieces, interleaved w1/w2 in phase A's consumption order; x rides
the gpsimd ring (first-group slices trickled early via deps, the rest
sequenced behind them); the scalar engine does nothing but gelu so
activations never stall; y stores go back on sync once weights finish.
"""

import numpy as np

B, T, D = 8, 768, 4096
N = B * T            # 6144 rows of x
NCORES = 8
TB = 512             # t-columns per core
P = 128
M0 = 133             # s-window starts this far before the t-block
SW = 784             # s-window width (133 + 512 + 133 + 6 pad)
NCH = 7              # s-chunks: 6 full + one 16-row chunk
LASTROWS = 16        # rows in chunk 6 (10 in-band + 6 pad)
DCH = D // P         # 32 d-chunks (contraction of phase A)
NQ = TB // P         # 4 t-subtiles per core
GN = 2048            # n-group width in phase B
NG = N // GN         # 3 n-groups
BANDCH = 4           # s-chunks feeding one t-subtile (covers +-133 band)
W1SPLIT = (2, 5, 5, 5, 5, 5, 5)   # w1 d-chunks per DMA piece

# Exact in-band t-range [lo_j, hi_j) of attn chunk j (window coords with
# M0 = 133: rows of chunk j are s - t0 + 133 in [128j, 128j+rows)).
BAND_LO = (0, 0, 0, 118, 246, 374, 502)
BAND_HI = (128, 256, 384, 512, 512, 512, 512)
BAND_W = tuple(h - l for l, h in zip(BAND_LO, BAND_HI))
MOFF = tuple(sum(BAND_W[:j]) for j in range(NCH))  # mask col offsets
MW = sum(BAND_W)  # 1576

_NC = None


def _build_module():
    from concourse import bacc, bass, mybir, tile
    from concourse.tile_rust import add_dep_helper

    f32 = mybir.dt.float32
    f16 = mybir.dt.float16
    PSUM = bass.MemorySpace.PSUM

    nc = bacc.Bacc("TRN2", target_bir_lowering=False, debug=False)
    xT_d = nc.declare_dram_parameter("xT_s", [NCH - 1, P, N], f16, isOutput=False)
    x6_d = nc.declare_dram_parameter("x6_s", [LASTROWS, N], f16, isOutput=False)
    w1_d = nc.declare_dram_parameter("w1_s", [P, DCH * SW], f16, isOutput=False)
    w2_d = nc.declare_dram_parameter("w2_s", [P, DCH * TB], f16, isOutput=False)
    mask_d = nc.declare_dram_parameter("mask_s", [P, MW], f16, isOutput=False)
    b2_d = nc.declare_dram_parameter("b2c_s", [P, NQ], f32, isOutput=False)
    yT_d = nc.declare_dram_parameter("yT_s", [TB, N], f16, isOutput=True)

    with tile.TileContext(nc) as tc:
        with (
            tc.tile_pool(name="const", bufs=1) as cpool,
            tc.tile_pool(name="attn", bufs=1) as apool,
            tc.tile_pool(name="xp", bufs=NG * NCH) as xp,
            tc.tile_pool(name="yp", bufs=4) as yp,
        ):
            b2_t = cpool.tile([P, NQ], f32)
            m_t = cpool.tile([P, MW], f16)

            # attn tiles are full 512 wide; zero them so phase B's 128-wide
            # stationary reads see zeros outside the exact band.
            attn_sb = []
            for j in range(NCH):
                a_t = apool.tile([P, TB], f16, name=f"attn_sb{j}")
                nc.vector.memset(a_t[:], 0.0)
                attn_sb.append(a_t)

            w_insts = []

            # ---- Phase A: attn[s, t] = (w1.T @ w2T) * mask on the band ----
            # Every DMA piece gets its OWN tile: consecutive DMAs into one
            # tile serialize (per-tile dependency tracking) and each pays a
            # ~2us completion bubble, killing the stream rate.
            with (
                tc.tile_pool(name="w1p", bufs=len(W1SPLIT)) as w1p,
                tc.tile_pool(name="w2p", bufs=DCH // 4) as w2p,
                tc.tile_pool(name="psA", bufs=1, space=PSUM) as psA,
            ):
                s1 = [int(v) for v in np.cumsum((0,) + W1SPLIT)]
                w1_ts = []
                w2_ts = []
                w1i = 0
                w2i = 0

                def w1_piece():
                    nonlocal w1i
                    t = w1p.tile([P, W1SPLIT[w1i] * SW], f16,
                                 name=f"w1_{w1i}", tag="w1")
                    w_insts.append(nc.sync.dma_start(
                        t[:], w1_d[:, s1[w1i] * SW:s1[w1i + 1] * SW]))
                    w1_ts.append(t)
                    w1i += 1

                def w2_piece():
                    nonlocal w2i
                    t = w2p.tile([P, 4 * TB], f16, name=f"w2_{w2i}", tag="w2")
                    w_insts.append(nc.scalar.dma_start(
                        t[:], w2_d[:, w2i * 4 * TB:(w2i + 1) * 4 * TB]))
                    w2_ts.append(t)
                    w2i += 1

                # issue order = w_insts index used by the x gates below
                w2_piece()             # 0: k0-3
                w1_piece()             # 1: k0-1
                w1_piece()             # 2: k2-6
                w2_piece()             # 3: k4-7
                w2_piece()             # 4: k8-11
                w1_piece()             # 5: k7-11
                w2_piece()             # 6: k12-15
                w1_piece()             # 7: k12-16
                w2_piece()             # 8: k16-19
                w1_piece()             # 9: k17-21
                w2_piece()             # 10: k20-23
                w1_piece()             # 11: k22-26
                w2_piece()             # 12: k24-27
                w1_piece()             # 13: k27-31
                w2_piece()             # 14: k28-31
                w_insts.append(nc.scalar.dma_start(m_t[:], mask_d[:]))
                w_insts.append(nc.scalar.dma_start(b2_t[:], b2_d[:]))

                def w1row_of(k):
                    pi = next(i for i in range(len(W1SPLIT))
                              if s1[i] <= k < s1[i + 1])
                    off = (k - s1[pi]) * SW
                    return w1_ts[pi][:, off:off + SW]

                attn_ps = [
                    psA.tile([P, BAND_W[j]], f32, name=f"attn_ps{j}")
                    for j in range(NCH)
                ]
                for k in range(DCH):
                    w1row = w1row_of(k)
                    w2row = w2_ts[k // 4][:, (k % 4) * TB:(k % 4 + 1) * TB]
                    # last k in j-ascending order so the mask-muls (and
                    # phase B's first subtile) unblock in consumption order
                    jorder = range(NCH) if k == DCH - 1 else (3, 2, 4, 1, 5, 0, 6)
                    for j in jorder:
                        scols = LASTROWS if j == 6 else P
                        nc.tensor.matmul(
                            attn_ps[j][0:scols, 0:BAND_W[j]],
                            w1row[:, j * P:j * P + scols],
                            w2row[:, BAND_LO[j]:BAND_HI[j]],
                            start=(k == 0),
                            stop=(k == DCH - 1),
                        )
                for j in range(NCH):
                    rows = LASTROWS if j == 6 else P
                    nc.vector.tensor_mul(
                        attn_sb[j][0:rows, BAND_LO[j]:BAND_HI[j]],
                        attn_ps[j][0:rows, 0:BAND_W[j]],
                        m_t[0:rows, MOFF[j]:MOFF[j] + BAND_W[j]],
                    )

            # x rides the gpsimd ring as per-(group, chunk) tiles.  Group-0
            # chunks are trickled during the weight phase via engine-stall
            # deps on the weight DMAs (the gpsimd trigger stream is FIFO,
            # so each gate also delays everything after it); groups 1-2
            # queue up behind them and drain during phase B.
            # Late release: the A-window is weight-DMA-bound while the
            # B-window has bandwidth slack, so x waits until the weight
            # stream is nearly done and still lands just in time.
            x_t = [[None] * NCH for _ in range(NG)]
            gates = {(0, 0): 11, (0, 1): 12, (0, 2): 12, (0, 3): 13,
                     (0, 4): 13, (0, 5): 14, (0, 6): 14, (1, 0): 16}
            for g in range(NG):
                for j in range(NCH):
                    rows = LASTROWS if j == 6 else P
                    xt = xp.tile([rows, GN], f16, name="x_t", tag="x_t")
                    src = x6_d if j == 6 else xT_d[j]
                    xi = nc.gpsimd.dma_start(
                        xt[:], src[:, g * GN:(g + 1) * GN])
                    gate = gates.get((g, j))
                    if gate is not None:
                        add_dep_helper(
                            xi.ins, w_insts[gate].ins, sync=True,
                            reason="pace x behind weights")
                    x_t[g][j] = xt

            # ---- Phase B: yT[t, n] = gelu(attn.T @ xT + b2) on the band ----
            with tc.tile_pool(name="psB", bufs=4, space=PSUM) as psB:
                for g in range(NG):
                    for q in range(NQ):
                        for h in range(2):
                            y_ps = psB.tile([P, 1024], f32, name="y_ps",
                                            tag="y_ps")
                            for hh in range(2):
                                osl = slice(hh * 512, (hh + 1) * 512)
                                xsl = slice(h * 1024 + hh * 512,
                                            h * 1024 + (hh + 1) * 512)
                                for c in range(BANDCH):
                                    j = q + c
                                    rows = LASTROWS if j == 6 else P
                                    nc.tensor.matmul(
                                        y_ps[:, osl],
                                        attn_sb[j][0:rows, q * P:(q + 1) * P],
                                        x_t[g][j][0:rows, xsl],
                                        start=(c == 0),
                                        stop=(c == BANDCH - 1),
                                    )
                            y_sb = yp.tile([P, 1024], f16, name="y_sb",
                                           tag="y_sb")
                            last = (g, q, h) == (NG - 1, NQ - 1, 1)
                            parts = (2 if last else 1)
                            for pp in range(parts):
                                lo = pp * 1024 // parts
                                hi = (pp + 1) * 1024 // parts
                                base = g * GN + h * 1024
                                nc.scalar.activation(
                                    y_sb[:, lo:hi],
                                    y_ps[:, lo:hi],
                                    mybir.ActivationFunctionType.Gelu,
                                    bias=b2_t[:, q:q + 1],
                                    scale=1.0,
                                )
                                nc.sync.dma_start(
                                    yT_d[q * P:(q + 1) * P,
                                         base + lo:base + hi],
                                    y_sb[:, lo:hi],
                                )

    nc.compile()
    nc.finalize()
    return nc


def _get_nc():
    global _NC
    if _NC is None:
        _NC = _build_module()
    return _NC


def prepare_in_maps(x, w1, w2, b2, sparse_mask):
    x = np.asarray(x, dtype=np.float32)
    w1 = np.asarray(w1, dtype=np.float32)
    w2 = np.asarray(w2, dtype=np.float32)
    b2 = np.asarray(b2, dtype=np.float32)
    sparse_mask = np.asarray(sparse_mask, dtype=np.float32)

    xT = np.ascontiguousarray(x.reshape(N, D).T.astype(np.float16))   # [s, n]
    w2T = np.ascontiguousarray(w2.T.astype(np.float16))               # [d, t]

    # Zero-pad the s axis (133 left, 139 right) so every core's window is
    # a plain slice; mask zeros make the padded rows contribute nothing.
    PADL, PADR = M0, SW - TB - M0
    xT_pad = np.zeros((D + PADL + PADR, N), dtype=np.float16)
    xT_pad[PADL:PADL + D] = xT
    w1_pad = np.zeros((D, D + PADL + PADR), dtype=np.float16)
    w1_pad[:, PADL:PADL + D] = w1.astype(np.float16)
    mask_pad = np.zeros((D + PADL + PADR, D), dtype=np.float16)
    mask_pad[PADL:PADL + D] = sparse_mask.astype(np.float16)

    in_maps = []
    for i in range(NCORES):
        s0 = i * TB           # window start in padded coords
        t0 = i * TB
        win = xT_pad[s0:s0 + SW]                          # [SW, N]
        # weights packed so each partition's 32 chunks are contiguous:
        # w1_s[p, k*SW + c] = w1_pad[128k + p, s0 + c]
        w1win = w1_pad[:, s0:s0 + SW]                     # [D, SW]
        w1_s = np.ascontiguousarray(
            w1win.reshape(DCH, P, SW).transpose(1, 0, 2).reshape(P, DCH * SW))
        w2win = w2T[:, t0:t0 + TB]                        # [D, TB]
        w2_s = np.ascontiguousarray(
            w2win.reshape(DCH, P, TB).transpose(1, 0, 2).reshape(P, DCH * TB))
        # mask packed per chunk at its exact band: [128, MW]
        mask_s = np.zeros((P, MW), dtype=np.float16)
        mwin = mask_pad[s0:s0 + SW, t0:t0 + TB]           # [SW, TB]
        for j in range(NCH):
            rows = LASTROWS if j == NCH - 1 else P
            mask_s[0:rows, MOFF[j]:MOFF[j] + BAND_W[j]] = (
                mwin[j * P:j * P + rows, BAND_LO[j]:BAND_HI[j]])
        in_maps.append({
            "xT_s": np.ascontiguousarray(
                win[:(NCH - 1) * P].reshape(NCH - 1, P, N)),
            "x6_s": np.ascontiguousarray(win[(NCH - 1) * P:]),
            "w1_s": w1_s,
            "w2_s": w2_s,
            "mask_s": mask_s,
            "b2c_s": np.ascontiguousarray(b2[t0:t0 + TB].reshape(NQ, P).T),
        })
    return in_maps


def assemble(results):
    out = np.empty((N, D), dtype=np.float32)
    for i in range(NCORES):
        out[:, i * TB:(i + 1) * TB] = results[i]["yT_s"].T.astype(np.float32)
    return out.reshape(B, T, D)


def _band_ok(sparse_mask):
    """The Bass kernel only computes attn on the exact per-chunk bands;
    verify every mask nonzero falls inside that region."""
    s_idx, t_idx = np.nonzero(np.asarray(sparse_mask) != 0)
    if len(s_idx) == 0:
        return True
    w0 = (t_idx // TB) * TB - M0              # per-core s-window start
    r = s_idx - w0                            # row in window coords
    if not np.all((r >= 0) & (r < SW - 6)):
        return False
    j = r // P
    tp = t_idx % TB
    lo = np.asarray(BAND_LO)[j]
    hi = np.asarray(BAND_HI)[j]
    return bool(np.all((tp >= lo) & (tp < hi)))


def _reference_fallback(x, w1, w2, b2, sparse_mask):
    import jax
    import jax.numpy as jnp

    cpu = jax.devices("cpu")[0]
    with jax.default_device(cpu):
        attn = jnp.einsum("ds,td->st", jnp.asarray(w1), jnp.asarray(w2))
        attn = attn * jnp.asarray(sparse_mask)
        y = jnp.einsum("bds,st->bdt", jnp.asarray(x), attn) + jnp.asarray(b2)
        return np.asarray(jax.nn.gelu(y, approximate=False), dtype=np.float32)


def kernel(x, w1, w2, b2, sparse_mask):
    import time

    from concourse.bass_utils import run_bass_kernel_spmd

    if (np.shape(x) != (B, T, D) or np.shape(w1) != (D, D)
            or np.shape(w2) != (D, D) or np.shape(b2) != (D,)
            or np.shape(sparse_mask) != (D, D) or not _band_ok(sparse_mask)):
        return _reference_fallback(x, w1, w2, b2, sparse_mask)

    in_maps = prepare_in_maps(x, w1, w2, b2, sparse_mask)
    nc = _get_nc()
    last_err = None
    for attempt in range(3):
        try:
            res = run_bass_kernel_spmd(nc, in_maps, list(range(NCORES)))
            return assemble(res.results)
        except Exception as e:  # transient NRT/device errors: retry
            last_err = e
            time.sleep(2.0 * (attempt + 1))
    raise last_err
